# revision 23
# baseline (speedup 1.0000x reference)
"""Trainium2 Bass kernel for a 2-layer GCN pair (BRIGHT arch) on 8 NeuronCores.

Layout: cores 0-3 process graph 1, cores 4-7 process graph 2 (one SPMD
program; per-core inputs differ). Within a 4-core group each core owns a
contiguous slice of SLICE node rows (node space padded to NP rows).

v2 design (gather-descgen-bound, so everything serves the Q7):
- Layer-1 gathers read a host-prescaled table xs = dinv*x directly (no
  on-device stage-1 table build); W1@W2 is folded into one host matmul
  W12 applied per dst tile at close. Layer-1 indices are signed int16
  against the table midpoint (one merged stream; xs is an input so the
  out-of-AP-region reads carry no scheduling hazard). The firmware trims
  trailing negative indices per dma_gather, so block ends are host-fixed
  to be non-negative.
- The one-hot S routing matrices are host-precomputed in fp8 and
  streamed from DRAM (PE matmul takes bf16 lhsT x fp8 rhs), eliminating
  the DVE is_equal builds that interfered with Q7 descriptor writes.
- Layer-2 uses three unsigned streams split at h2-exchange chunk
  boundaries with exact in_ap regions, so each stream's gathers start as
  soon as its AllGather chunks land (pipelined against layer-1 closes;
  chunk-major h2full layout via host-permuted layer-2 indices).
- dma_gather blocks rotate across all 4 SWDGE queues.
"""

import numpy as np
import ml_dtypes

import concourse.bass as bass
import concourse.tile as tile
from concourse import bacc, mybir
from concourse.bass_utils import run_bass_kernel_spmd

F32 = mybir.dt.float32
BF16 = mybir.dt.bfloat16
FP8 = mybir.dt.float8e4
I16 = mybir.dt.int16

EPS = 1e-12
GC = 6   # gather-block size in chunks (GC*128 indices per dma_gather)
SP = True  # single-packet drains (safe at <=64+1 descs/engine, i.e. GC<=8)
S_MODE = "dve"  # "stream": host-built fp8 S from DRAM; "dve": is_equal builds
NQ = 4   # SWDGE queues
GATHER_QS = (0, 1, 2, 3)

_prog_cache: dict = {}

bf = ml_dtypes.bfloat16
f8 = ml_dtypes.float8_e4m3


# ---------------------------------------------------------------- host prep

def _chunk_bounds(TLOC):
    """Tile-index boundaries for the chunked h~2 AllGather pipeline."""
    if TLOC < 8:
        mid = (TLOC + 1) // 2
        return [0, mid, TLOC]
    fr = [0.0, 0.26, 0.52, 0.72, 0.88, 0.96, 1.0]
    b = sorted(set(int(round(f * TLOC)) for f in fr))
    return b


def _perm_from_bounds(bounds, SLICE, NP, group_size):
    """Node id -> row in the chunk-major h2full table."""
    p = np.empty(NP, np.int64)
    for s, e in zip(bounds[:-1], bounds[1:]):
        rs, re = s * 128, e * 128
        n_rows = re - rs
        for g in range(group_size):
            lo = g * SLICE + rs
            p[lo:lo + n_rows] = (group_size * rs + g * n_rows
                                 + np.arange(n_rows))
    return p


def _pack_idx(flat):
    """slot j -> wrapped [16, n/16] then replicated to [128, n/16] int16."""
    n = flat.shape[0]
    assert n % 16 == 0
    w = flat.reshape(n // 16, 16).T.astype(np.int16)
    return np.tile(w, (8, 1))


def _prep_graph(edge_index, N, NP, SLICE, n_cores, ranges, src_map=None):
    """Per-core, per-tile, per-range edge buckets for one graph.

    ranges: list of (lo, hi, base) row intervals partitioning [0, NP); an
    edge goes to the range containing its (mapped) src row, with index
    row - base. Returns (dinv, dinvsq, cores) with
    cores[c][t][r] = (idx_rel, dloc), idx_rel ascending.
    """
    src = np.asarray(edge_index[0], dtype=np.int64)
    dst = np.asarray(edge_index[1], dtype=np.int64)
    deg = np.bincount(dst, minlength=N).astype(np.float64) + 1.0
    dinv = (1.0 / np.sqrt(deg)).astype(np.float32)
    dinvsq = (1.0 / deg).astype(np.float32)

    rows = src_map[src] if src_map is not None else src
    for lo, hi, base in ranges:
        assert base - lo <= 32768 and hi - base <= 32768, (lo, hi, base)

    TLOC = SLICE // 128
    cores = []
    for c in range(n_cores):
        clo, chi = c * SLICE, (c + 1) * SLICE
        sel = (dst >= clo) & (dst < chi)
        s = rows[sel]
        d = dst[sel] - clo
        t_id = d // 128
        dloc = d % 128
        order = np.argsort(t_id, kind="stable")
        s, dloc, t_id = s[order], dloc[order], t_id[order]
        tiles = []
        for t in range(TLOC):
            m = t_id == t
            st, dt_ = s[m], dloc[m]
            bks = []
            for lo, hi, base in ranges:
                mr = (st >= lo) & (st < hi)
                sr = st[mr] - base
                dr = dt_[mr]
                o = np.argsort(sr, kind="stable")
                bks.append((sr[o].astype(np.int64), dr[o]))
            tiles.append(bks)
        cores.append(tiles)
    return dinv, dinvsq, cores


def _slot_counts(graph_cores_list, TLOC, R, min_one):
    """Shared per-tile per-range chunk counts (max across all datasets)."""
    K = [np.zeros(TLOC, np.int64) for _ in range(R)]
    for cores in graph_cores_list:
        for tiles in cores:
            for t in range(TLOC):
                for r in range(R):
                    n = len(tiles[t][r][0])
                    K[r][t] = max(K[r][t], (n + 127) // 128)
    for r in range(R):
        if min_one[r]:
            K[r] = np.maximum(K[r], 1)
    return K


def _build_stream(tiles, K, TLOC, r):
    """One core's range-r stream: padded idx/S arrays + block-end fix."""
    idx_l, dl_l, tile_ends = [], [], []
    pos = 0
    for t in range(TLOC):
        s_, d_ = tiles[t][r]
        slots = K[t] * 128
        n = s_.shape[0]
        assert n <= slots, (n, slots)
        si = np.zeros(slots, np.int64)
        di = np.full(slots, -1.0, np.float32)
        si[:n] = s_
        di[:n] = d_.astype(np.float32)
        idx_l.append(si)
        dl_l.append(di)
        pos += slots
        tile_ends.append(pos)
    if not idx_l or pos == 0:
        return (np.zeros((128, 1), np.int16),
                np.zeros((128, 1), np.uint8).view(f8),
                np.zeros((128, 1), np.float32).astype(bf))
    idx = np.concatenate(idx_l)
    dl = np.concatenate(dl_l)
    Ltot = idx.shape[0]

    # firmware trims trailing negative idxs per dma_gather: make sure the
    # last slot of every gather block is non-negative. Swap each negative
    # block-end slot with a distinct non-negative slot of the same dst
    # tile (slot order within a tile is free; S follows the final order).
    ends = list(range(GC * 128 - 1, Ltot, GC * 128))
    if not ends or ends[-1] != Ltot - 1:
        ends.append(Ltot - 1)
    ends_set = set(ends)
    te = np.asarray(tile_ends)
    ts = te - np.asarray([te[0]] + list(np.diff(te)))
    for t in range(len(te)):
        t0, t1 = int(ts[t]), int(te[t])
        needy = [p for p in ends if t0 <= p < t1 and idx[p] < 0]
        if not needy:
            continue
        donors = (q for q in range(t1 - 1, t0 - 1, -1)
                  if idx[q] >= 0 and q not in ends_set)
        for p in needy:
            q = next(donors)
            idx[p], idx[q] = idx[q], idx[p]
            dl[p], dl[q] = dl[q], dl[p]

    # one-hot S in fp8 bytes: [slot(128), chunk, 128]; dl as [slot, chunk]
    C = Ltot // 128
    dlm = dl.reshape(C, 128).astype(np.int32)
    if S_MODE == "stream":
        S = (dlm[:, :, None] == np.arange(128, dtype=np.int32))
        S = (S.astype(np.uint8) * 0x38).transpose(1, 0, 2).reshape(128, C * 128)
        S = np.ascontiguousarray(S).view(f8)
    else:
        S = np.zeros((128, 1), np.uint8).view(f8)
    dlc = np.ascontiguousarray(dl.reshape(C, 128).T).astype(bf)
    return _pack_idx(idx), S, dlc


def _cols_from_vec(v_padded, TL):
    """[TL*128] -> [128, TL] per-tile columns."""
    return np.ascontiguousarray(v_padded.reshape(TL, 128).T)


# ---------------------------------------------------------------- builder

def _build_program(NP, SLICE, l2rows, K1, K2, bounds,
                   n_cores_total, group_size):
    TLOC = SLICE // 128
    NL2 = len(K2)
    C1 = int(sum(K1))
    C2 = [int(sum(k)) for k in K2]
    MID1 = NP // 2

    nc = bacc.Bacc("TRN2", target_bir_lowering=False, debug=False,
                   num_devices=n_cores_total, num_swdge_queues=NQ,
                   dynamic_dma_scratch_size=65536)

    _sw = (lambda C: max(C * 128, 1)) if S_MODE == "stream" else (lambda C: 1)
    xs = nc.dram_tensor("xs", [NP, 128], BF16, kind="ExternalInput")
    rwrT = nc.dram_tensor("rwrT", [128, SLICE], BF16, kind="ExternalInput")
    xTloc = nc.dram_tensor("xTloc", [128, SLICE], BF16, kind="ExternalInput")
    idx1 = nc.dram_tensor("idx1", [128, max(C1 * 8, 1)], I16, kind="ExternalInput")
    S1 = nc.dram_tensor("S1", [128, _sw(C1)], FP8, kind="ExternalInput")
    idx2 = [nc.dram_tensor(f"idx2{r}", [128, max(C2[r] * 8, 1)], I16,
                           kind="ExternalInput") for r in range(NL2)]
    S2 = [nc.dram_tensor(f"S2{r}", [128, _sw(C2[r])], FP8,
                         kind="ExternalInput") for r in range(NL2)]
    dl1 = nc.dram_tensor("dl1", [128, max(C1, 1)], BF16, kind="ExternalInput")
    dl2 = [nc.dram_tensor(f"dl2{r}", [128, max(C2[r], 1)], BF16,
                          kind="ExternalInput") for r in range(NL2)]
    iota = nc.dram_tensor("iota", [128, 128], BF16, kind="ExternalInput")
    dinv_loc = nc.dram_tensor("dinv_loc", [128, TLOC], F32, kind="ExternalInput")
    dinvsq_loc = nc.dram_tensor("dinvsq_loc", [128, TLOC], F32, kind="ExternalInput")
    W12 = nc.dram_tensor("W12", [128, 128], BF16, kind="ExternalInput")
    linW = nc.dram_tensor("linW", [128, 128], BF16, kind="ExternalInput")
    combWt = nc.dram_tensor("combWt", [128, 128], BF16, kind="ExternalInput")
    combWb = nc.dram_tensor("combWb", [128, 128], BF16, kind="ExternalInput")
    ident = nc.dram_tensor("ident", [128, 128], BF16, kind="ExternalInput")
    emd_out = nc.dram_tensor("emd_out", [SLICE, 128], F32, kind="ExternalOutput")

    groups = [
        list(range(g * group_size, (g + 1) * group_size))
        for g in range(n_cores_total // group_size)
    ]

    with tile.TileContext(nc) as tc:
        with tc.tile_pool(name="dram", bufs=1, space="DRAM") as dram, \
             tc.tile_pool(name="const", bufs=1) as cp, \
             tc.tile_pool(name="blkA", bufs=5) as bap, \
             tc.tile_pool(name="blkB", bufs=3) as bbp, \
             tc.tile_pool(name="sA", bufs=5) as sap, \
             tc.tile_pool(name="sB", bufs=3) as sbp, \
             tc.tile_pool(name="work", bufs=3) as wp, \
             tc.tile_pool(name="norm", bufs=4) as npools, \
             tc.tile_pool(name="ps_agg", bufs=3, space="PSUM") as ps_agg, \
             tc.tile_pool(name="ps_aux", bufs=2, space="PSUM") as ps_aux, \
             tc.tile_pool(name="ps_tr", bufs=2, space="PSUM") as ps_tr:

            h2slice = dram.tile([SLICE, 128], BF16)
            h2full = dram.tile([NP, 128], BF16)
            posT_d = dram.tile([SLICE, 128], BF16)

            def cload(t_dram, shape, dt, tag):
                t_sb = cp.tile(shape, dt, tag=tag)
                nc.sync.dma_start(t_sb[:], t_dram[:, :])
                return t_sb

            idx1_t = cload(idx1, [128, max(C1 * 8, 1)], I16, "idx1")
            idx2_t = [cload(idx2[r], [128, max(C2[r] * 8, 1)], I16, f"idx2{r}")
                      for r in range(NL2)]
            dl1_t = cload(dl1, [128, max(C1, 1)], BF16, "dl1")
            dl2_t = [cload(dl2[r], [128, max(C2[r], 1)], BF16, f"dl2{r}")
                     for r in range(NL2)]
            iota_t = cload(iota, [128, 128], BF16, "iota")
            dinvl_t = cload(dinv_loc, [128, TLOC], F32, "dinvl")
            dinvsq_t = cload(dinvsq_loc, [128, TLOC], F32, "dinvsq")
            W12_t = cload(W12, [128, 128], BF16, "W12")
            linW_t = cload(linW, [128, 128], BF16, "linW")
            combWt_t = cload(combWt, [128, 128], BF16, "combWt")
            combWb_t = cload(combWb, [128, 128], BF16, "combWb")
            ident_t = cload(ident, [128, 128], BF16, "ident")

            Copy = mybir.ActivationFunctionType.Copy

            def l1norm_scale(src_ap, out_tile_ap):
                """out = src / max(sum|src|, EPS), per-partition rows."""
                s_sum = npools.tile([128, 1], F32, tag="nsum")
                nc.vector.reduce_sum(
                    s_sum[:], src_ap, axis=mybir.AxisListType.X,
                    apply_absolute_value=True)
                s_max = npools.tile([128, 1], F32, tag="nmax")
                nc.vector.tensor_scalar_max(s_max[:], s_sum[:], EPS)
                r = npools.tile([128, 1], F32, tag="nrec")
                nc.vector.reciprocal(r[:], s_max[:])
                nc.scalar.activation(out_tile_ap, src_ap, Copy, scale=r[:, 0:1])

            qctr = [0]

            def agg_pass(streams, node_major):
                """Chunked aggregation over all local tiles.

                streams: list of (idx_sb, S_dram, dl_sb, table_ap, K, CT,
                bpool, spool, tag). node_major False: psum[f, dst]
                (lhsT=Hg, rhs=S); True: psum[dst, f] (lhsT=S, rhs=Hg).
                Yields (t, psum_tile) at each tile close.
                """
                issued = [0] * len(streams)
                blocks = [dict() for _ in streams]
                qpos = [0] * len(streams)

                def issue_block(r):
                    (idx_t, S_d, dl_t, table_ap, K, CT, bpool, spool,
                     tag) = streams[r]
                    b = issued[r]
                    q0 = b * GC
                    if q0 >= CT:
                        return
                    cb = min(GC, CT - q0)
                    blk = bpool.tile([128, GC, 128], BF16, tag="b" + tag)
                    nc.gpsimd.dma_gather(
                        blk[:, :cb, :], table_ap,
                        idx_t[:, q0 * 8:(q0 + cb) * 8],
                        num_idxs=cb * 128, num_idxs_reg=cb * 128,
                        elem_size=128, single_packet=SP,
                        queue_num=GATHER_QS[qctr[0] % len(GATHER_QS)])
                    qctr[0] += 1
                    sblk = spool.tile([128, GC, 128], FP8, tag="s" + tag)
                    if S_MODE == "stream":
                        nc.scalar.dma_start(
                            sblk[:, :cb, :],
                            S_d[:, q0 * 128:(q0 + cb) * 128]
                            .rearrange("p (c d) -> p c d", c=cb))
                    else:
                        nc.vector.tensor_tensor(
                            out=sblk[:, :cb, :],
                            in0=iota_t[:].unsqueeze(1)
                                .broadcast_to([128, cb, 128]),
                            in1=dl_t[:, q0:q0 + cb].unsqueeze(2)
                                .broadcast_to([128, cb, 128]),
                            op=mybir.AluOpType.is_equal)
                    blocks[r].pop(b - 5, None)
                    blocks[r][b] = (blk, sblk)
                    issued[r] = b + 1

                for t in range(TLOC):
                    ps = ps_agg.tile([128, 128], F32, tag="agg")
                    done = 0
                    for r, st in enumerate(streams):
                        K = st[4]
                        q = qpos[r]
                        for i in range(K[t]):
                            while issued[r] * GC <= q:
                                issue_block(r)
                            blk, sblk = blocks[r][q // GC]
                            s_t = sblk[:, q % GC, :]
                            hg = blk[:, q % GC, :]
                            if node_major:
                                nc.tensor.matmul(ps[:], lhsT=s_t, rhs=hg,
                                                 start=(done == 0), stop=False)
                            else:
                                nc.tensor.matmul(ps[:], lhsT=hg, rhs=s_t,
                                                 start=(done == 0), stop=False)
                            q += 1
                            done += 1
                        qpos[r] = q
                    yield t, ps

            # ================= layer 1: feature-major agg of xs -> h~2 slice,
            # with the group AllGather pipelined chunk-by-chunk
            cc_next = 0
            st1 = [(idx1_t, S1, dl1_t, xs[MID1:NP, :], K1, C1, bap, sap,
                    "1")]
            for t, ps in agg_pass(st1, False):
                # self term (pre-W12): += ident^T @ (dinv*X_T)[:, own tile]
                xl = wp.tile([128, 128], BF16, tag="xl")
                nc.sync.dma_start(xl[:], xTloc[:, t * 128:(t + 1) * 128])
                nc.tensor.matmul(ps[:], lhsT=ident_t[:], rhs=xl[:],
                                 start=False, stop=True)
                # close: M[f, dst] -> h~2 tile = dinvsq * (W12^T M)^T
                M_sb = wp.tile([128, 128], BF16, tag="aggT")
                nc.scalar.activation(M_sb[:], ps[:], Copy)
                h2T_ps = ps_aux.tile([128, 128], F32, tag="mm")
                nc.tensor.matmul(h2T_ps[:], lhsT=W12_t[:], rhs=M_sb[:],
                                 start=True, stop=True)
                h2T_sb = wp.tile([128, 128], BF16, tag="h2Ts")
                nc.scalar.activation(h2T_sb[:], h2T_ps[:], Copy)
                h2_ps = ps_tr.tile([128, 128], BF16, tag="tr")
                nc.tensor.transpose(h2_ps[:], h2T_sb[:], ident_t[:])
                h2_sb = wp.tile([128, 128], BF16, tag="h2s")
                nc.scalar.activation(h2_sb[:], h2_ps[:], Copy,
                                     scale=dinvsq_t[:, t:t + 1])
                nc.sync.dma_start(h2slice[t * 128:(t + 1) * 128, :], h2_sb[:])
                # pos = l1norm(rwr @ linW), transposed; input-only, done here
                rw = wp.tile([128, 128], BF16, tag="rw")
                nc.sync.dma_start(rw[:], rwrT[:, t * 128:(t + 1) * 128])
                pos_ps = ps_aux.tile([128, 128], F32, tag="mm")
                nc.tensor.matmul(pos_ps[:], lhsT=rw[:], rhs=linW_t[:],
                                 start=True, stop=True)
                pos_bf = wp.tile([128, 128], BF16, tag="posbf")
                l1norm_scale(pos_ps[:], pos_bf[:])
                posT_ps = ps_tr.tile([128, 128], BF16, tag="tr")
                nc.tensor.transpose(posT_ps[:], pos_bf[:], ident_t[:])
                posT_sb = wp.tile([128, 128], BF16, tag="posT")
                nc.scalar.activation(posT_sb[:], posT_ps[:], Copy)
                nc.sync.dma_start(posT_d[t * 128:(t + 1) * 128, :],
                                  posT_sb[:])
                if t + 1 == bounds[cc_next + 1]:
                    rs, re = bounds[cc_next] * 128, bounds[cc_next + 1] * 128
                    nc.gpsimd.collective_compute(
                        "AllGather", mybir.AluOpType.bypass,
                        replica_groups=groups,
                        ins=[h2slice[rs:re, :].opt()],
                        outs=[h2full[group_size * rs:group_size * re, :].opt()])
                    cc_next += 1

            # ================= layer 2: node-major agg + head
            st2 = [
                (idx2_t[r], S2[r], dl2_t[r],
                 h2full[l2rows[r]:l2rows[r + 1], :],
                 K2[r], C2[r], (bap if r < 2 else bbp),
                 (sap if r < 2 else sbp), f"2{r}")
                for r in range(NL2)
            ]
            for t, ps in agg_pass(st2, True):
                # self-loop term: += h~2[own tile] (identity matmul)
                h2s = wp.tile([128, 128], BF16, tag="h2self")
                nc.sync.dma_start(h2s[:], h2slice[t * 128:(t + 1) * 128, :])
                nc.tensor.matmul(ps[:], lhsT=ident_t[:], rhs=h2s[:],
                                 start=False, stop=True)
                # g = l1norm(dinv * agg2)
                g_pre = wp.tile([128, 128], F32, tag="gpre")
                nc.scalar.activation(g_pre[:], ps[:], Copy,
                                     scale=dinvl_t[:, t:t + 1])
                g_bf = wp.tile([128, 128], BF16, tag="gbf")
                l1norm_scale(g_pre[:], g_bf[:])
                gT_ps = ps_tr.tile([128, 128], BF16, tag="tr")
                nc.tensor.transpose(gT_ps[:], g_bf[:], ident_t[:])
                gT_sb = wp.tile([128, 128], BF16, tag="gT")
                nc.scalar.activation(gT_sb[:], gT_ps[:], Copy)

                # emd = l1norm(concat(pos, g) @ combW); posT precomputed
                posT_sb = wp.tile([128, 128], BF16, tag="posT")
                nc.sync.dma_start(posT_sb[:],
                                  posT_d[t * 128:(t + 1) * 128, :])

                emd_ps = ps_aux.tile([128, 128], F32, tag="mm")
                nc.tensor.matmul(emd_ps[:], lhsT=posT_sb[:], rhs=combWt_t[:],
                                 start=True, stop=False)
                nc.tensor.matmul(emd_ps[:], lhsT=gT_sb[:], rhs=combWb_t[:],
                                 start=False, stop=True)
                emd_f = wp.tile([128, 128], F32, tag="emdf")
                l1norm_scale(emd_ps[:], emd_f[:])
                nc.sync.dma_start(emd_out[t * 128:(t + 1) * 128, :], emd_f[:])

    nc.compile()
    return nc


# ---------------------------------------------------------------- kernel

def _l2_rows(bounds, group_size, TLOC):
    """Layer-2 stream row boundaries: exchange-chunk-aligned spans of
    <= 32768 rows each; the first two exchange chunks get their own
    streams so layer-2 gathers start right after the first exchange."""
    rows = [group_size * 128 * b for b in bounds]
    NP = rows[-1]
    cuts = [rows[0]]
    for i, r in enumerate(rows[1:], 1):
        nxt = rows[i + 1] if i + 1 < len(rows) else None
        if len(cuts) < 3 and r - cuts[-1] > 0 and r < NP:
            cuts.append(r)
        elif nxt is None:
            cuts.append(r)
        elif nxt - cuts[-1] > 32768:
            cuts.append(r)
    if cuts[-1] != NP:
        cuts.append(NP)
    cuts = sorted(set(cuts))
    for a, b in zip(cuts[:-1], cuts[1:]):
        assert 0 < b - a <= 32768, (cuts, a, b)
    return cuts


def _run(inputs, N, E, n_cores_total=8, group_size=4):
    n_groups = n_cores_total // group_size
    assert n_groups == 2
    SLICE = ((N + group_size * 128 - 1) // (group_size * 128)) * 128
    NP = SLICE * group_size
    TLOC = SLICE // 128

    bounds = _chunk_bounds(TLOC)
    perm = _perm_from_bounds(bounds, SLICE, NP, group_size)
    l2rows = _l2_rows(bounds, group_size, TLOC)

    MID1 = NP // 2
    ranges1 = [(0, NP, MID1)]
    NL2 = len(l2rows) - 1
    ranges2 = [(l2rows[r], l2rows[r + 1], l2rows[r]) for r in range(NL2)]

    graphs = []
    for g in range(2):
        ei = inputs["edge_index1" if g == 0 else "edge_index2"]
        dinv, dinvsq, cores = _prep_graph(ei, N, NP, SLICE, group_size,
                                          ranges1)
        _, _, cores2 = _prep_graph(ei, N, NP, SLICE, group_size,
                                   ranges2, src_map=perm)
        graphs.append((dinv, dinvsq, cores, cores2))

    (K1,) = _slot_counts([g[2] for g in graphs], TLOC, 1, [True])
    K2 = _slot_counts([g[3] for g in graphs], TLOC, NL2,
                      [True] + [False] * (NL2 - 1))

    key = (NP, SLICE, tuple(l2rows), tuple(K1),
           tuple(tuple(k) for k in K2),
           tuple(bounds), n_cores_total, group_size)
    if key not in _prog_cache:
        _prog_cache[key] = _build_program(
            NP, SLICE, l2rows, K1, K2, bounds, n_cores_total, group_size)
    nc = _prog_cache[key]

    ident_np = np.eye(128, dtype=np.float32).astype(bf)
    W1f = np.asarray(inputs["conv1_W"], np.float32)
    W2f = np.asarray(inputs["conv2_W"], np.float32)
    W12_np = (W1f @ W2f).astype(bf)
    linW_np = np.asarray(inputs["lin_W"], np.float32).astype(bf)
    combW = np.asarray(inputs["comb_W"], np.float32)
    combWt_np = combW[:128].astype(bf)
    combWb_np = combW[128:].astype(bf)

    in_maps = []
    for core in range(n_cores_total):
        g = core // group_size
        c = core % group_size
        dinv, dinvsq, cores, cores2 = graphs[g]
        x = np.asarray(inputs["x1" if g == 0 else "x2"], np.float32)
        rwr = np.asarray(inputs["rwr1_emd" if g == 0 else "rwr2_emd"],
                         np.float32)

        dinv_p = np.ones(NP, np.float32)
        dinv_p[:N] = dinv
        dinvsq_p = np.ones(NP, np.float32)
        dinvsq_p[:N] = dinvsq

        xs = np.zeros((NP, 128), np.float32)
        xs[:N] = x * dinv[:, None]
        rwrT = np.zeros((128, SLICE), np.float32)
        lo, hi = c * SLICE, min((c + 1) * SLICE, N)
        if hi > lo:
            rwrT[:, :hi - lo] = rwr[lo:hi].T
        sl = slice(c * SLICE, (c + 1) * SLICE)
        xTloc = np.zeros((128, SLICE), np.float32)
        if hi > lo:
            xTloc[:, :hi - lo] = (x[lo:hi] * dinv[lo:hi, None]).T

        i1, s1, d1 = _build_stream(cores[c], K1, TLOC, 0)
        im = {
            "xs": xs.astype(bf),
            "rwrT": rwrT.astype(bf),
            "xTloc": xTloc.astype(bf),
            "idx1": i1, "S1": s1, "dl1": d1,
            "iota": np.broadcast_to(
                np.arange(128, dtype=np.float32), (128, 128)).astype(bf),
            "dinv_loc": _cols_from_vec(dinv_p[sl], TLOC),
            "dinvsq_loc": _cols_from_vec(dinvsq_p[sl], TLOC),
            "W12": W12_np, "linW": linW_np,
            "combWt": combWt_np, "combWb": combWb_np,
            "ident": ident_np,
        }
        for r in range(NL2):
            i2, s2, d2 = _build_stream(cores2[c], K2[r], TLOC, r)
            im[f"idx2{r}"] = i2
            im[f"S2{r}"] = s2
            im[f"dl2{r}"] = d2
        in_maps.append(im)

    import os
    if os.environ.get("GCN_SIM"):
        from concourse.bass_interp import MultiCoreSim
        sim = MultiCoreSim(nc, num_cores=n_cores_total, trace=False,
                           require_finite=False, require_nnan=False)
        cores = list(sim.cores.values())
        for c, core_sim in enumerate(cores):
            for k, v in in_maps[c].items():
                core_sim.tensor(k)[:] = v
        sim.simulate(check_with_hw=False)

        class _R:
            results = [{"emd_out": np.array(core_sim.tensor("emd_out"))}
                       for core_sim in cores]
        res = _R()
    else:
        trace = bool(os.environ.get("GCN_TRACE"))
        if trace:
            import sys, types
            if "antenv.axon_hooks" not in sys.modules:
                mod = types.ModuleType("antenv.axon_hooks")
                mod._hook = None
                mod.set_axon_ntff_profile_hook = \
                    lambda h: setattr(mod, "_hook", h)
                mod.get_axon_ntff_profile_hook = lambda: mod._hook
                sys.modules["antenv.axon_hooks"] = mod
                from trn_agent_boot.trn_boot import _ntff_profile_via_ctypes
                mod.set_axon_ntff_profile_hook(
                    _ntff_profile_via_ctypes('/opt/axon/libaxon_pjrt.so'))
        res = run_bass_kernel_spmd(nc, in_maps,
                                   core_ids=list(range(n_cores_total)),
                                   trace=trace)
        if trace:
            print(f"HW exec time: {res.exec_time_ns} ns "
                  f"(mean {res.mean_exec_time_ns}, "
                  f"core {res.max_exec_time_core_id})")
            if res.instructions_and_trace:
                print("trace:", res.instructions_and_trace[1])

    outs = []
    for g in range(2):
        parts = [res.results[g * group_size + c]["emd_out"]
                 for c in range(group_size)]
        outs.append(np.concatenate(parts, axis=0)[:N])
    return outs[0], outs[1]


def kernel(rwr1_emd, rwr2_emd, x1, x2, edge_index1, edge_index2,
           lin_W, lin_b, conv1_W, conv1_b, conv2_W, conv2_b,
           comb_W, comb_b):
    for name, b in (("lin_b", lin_b), ("conv1_b", conv1_b),
                    ("conv2_b", conv2_b), ("comb_b", comb_b)):
        if np.any(np.asarray(b) != 0):
            raise NotImplementedError(f"nonzero bias {name} not supported")
    inputs = dict(rwr1_emd=rwr1_emd, rwr2_emd=rwr2_emd, x1=x1, x2=x2,
                  edge_index1=edge_index1, edge_index2=edge_index2,
                  lin_W=lin_W, conv1_W=conv1_W, conv2_W=conv2_W,
                  comb_W=comb_W)
    N = np.asarray(x1).shape[0]
    E = np.asarray(edge_index1).shape[1]
    return _run(inputs, N, E)


# revision 25
# speedup vs baseline: 1.2067x; 1.2067x over previous
"""Trainium2 Bass kernel for a 2-layer GCN pair (BRIGHT arch) on 8 NeuronCores.

Layout: cores 0-3 process graph 1, cores 4-7 process graph 2 (one SPMD
program; per-core inputs differ). Within a 4-core group each core owns a
contiguous slice of SLICE node rows (node space padded to NP rows).

v2 design (gather-descgen-bound, so everything serves the Q7):
- Layer-1 gathers read a host-prescaled table xs = dinv*x directly (no
  on-device stage-1 table build); W1@W2 is folded into one host matmul
  W12 applied per dst tile at close. Layer-1 indices are signed int16
  against the table midpoint (one merged stream; xs is an input so the
  out-of-AP-region reads carry no scheduling hazard). The firmware trims
  trailing negative indices per dma_gather, so block ends are host-fixed
  to be non-negative.
- The one-hot S routing matrices are host-precomputed in fp8 and
  streamed from DRAM (PE matmul takes bf16 lhsT x fp8 rhs), eliminating
  the DVE is_equal builds that interfered with Q7 descriptor writes.
- Layer-2 uses three unsigned streams split at h2-exchange chunk
  boundaries with exact in_ap regions, so each stream's gathers start as
  soon as its AllGather chunks land (pipelined against layer-1 closes;
  chunk-major h2full layout via host-permuted layer-2 indices).
- dma_gather blocks rotate across all 4 SWDGE queues.
"""

import numpy as np
import ml_dtypes

import concourse.bass as bass
import concourse.tile as tile
from concourse import bacc, mybir
from concourse.bass_utils import run_bass_kernel_spmd

F32 = mybir.dt.float32
BF16 = mybir.dt.bfloat16
FP8 = mybir.dt.float8e4
I16 = mybir.dt.int16

EPS = 1e-12
GC = 8   # gather-block size in chunks (GC*128 indices per dma_gather)
SP = True  # single-packet drains (safe at <=64+1 descs/engine, i.e. GC<=8)
S_MODE = "dve"  # "stream": host-built fp8 S from DRAM; "dve": is_equal builds
NQ = 4   # SWDGE queues
GATHER_QS = (0, 1, 2, 3)

_prog_cache: dict = {}

bf = ml_dtypes.bfloat16
f8 = ml_dtypes.float8_e4m3


# ---------------------------------------------------------------- host prep

def _chunk_bounds(TLOC):
    """Tile-index boundaries for the chunked h~2 AllGather pipeline."""
    if TLOC < 8:
        mid = (TLOC + 1) // 2
        return [0, mid, TLOC]
    fr = [0.0, 0.26, 0.52, 0.72, 0.88, 0.96, 1.0]
    b = sorted(set(int(round(f * TLOC)) for f in fr))
    return b


def _perm_from_bounds(bounds, SLICE, NP, group_size):
    """Node id -> row in the chunk-major h2full table."""
    p = np.empty(NP, np.int64)
    for s, e in zip(bounds[:-1], bounds[1:]):
        rs, re = s * 128, e * 128
        n_rows = re - rs
        for g in range(group_size):
            lo = g * SLICE + rs
            p[lo:lo + n_rows] = (group_size * rs + g * n_rows
                                 + np.arange(n_rows))
    return p


def _pack_idx(flat):
    """slot j -> wrapped [16, n/16] then replicated to [128, n/16] int16."""
    n = flat.shape[0]
    assert n % 16 == 0
    w = flat.reshape(n // 16, 16).T.astype(np.int16)
    return np.tile(w, (8, 1))


def _prep_graph(edge_index, N, NP, SLICE, n_cores, ranges, src_map=None):
    """Per-core, per-tile, per-range edge buckets for one graph.

    ranges: list of (lo, hi, base) row intervals partitioning [0, NP); an
    edge goes to the range containing its (mapped) src row, with index
    row - base. Returns (dinv, dinvsq, cores) with
    cores[c][t][r] = (idx_rel, dloc), idx_rel ascending.
    """
    src = np.asarray(edge_index[0], dtype=np.int64)
    dst = np.asarray(edge_index[1], dtype=np.int64)
    deg = np.bincount(dst, minlength=N).astype(np.float64) + 1.0
    dinv = (1.0 / np.sqrt(deg)).astype(np.float32)
    dinvsq = (1.0 / deg).astype(np.float32)

    rows = src_map[src] if src_map is not None else src
    for lo, hi, base in ranges:
        assert base - lo <= 32768 and hi - base <= 32768, (lo, hi, base)

    TLOC = SLICE // 128
    cores = []
    for c in range(n_cores):
        clo, chi = c * SLICE, (c + 1) * SLICE
        sel = (dst >= clo) & (dst < chi)
        s = rows[sel]
        d = dst[sel] - clo
        t_id = d // 128
        dloc = d % 128
        order = np.argsort(t_id, kind="stable")
        s, dloc, t_id = s[order], dloc[order], t_id[order]
        tiles = []
        for t in range(TLOC):
            m = t_id == t
            st, dt_ = s[m], dloc[m]
            bks = []
            for lo, hi, base in ranges:
                mr = (st >= lo) & (st < hi)
                sr = st[mr] - base
                dr = dt_[mr]
                o = np.argsort(sr, kind="stable")
                bks.append((sr[o].astype(np.int64), dr[o]))
            tiles.append(bks)
        cores.append(tiles)
    return dinv, dinvsq, cores


def _slot_counts(graph_cores_list, TLOC, R, min_one):
    """Shared per-tile per-range chunk counts (max across all datasets)."""
    K = [np.zeros(TLOC, np.int64) for _ in range(R)]
    for cores in graph_cores_list:
        for tiles in cores:
            for t in range(TLOC):
                for r in range(R):
                    n = len(tiles[t][r][0])
                    K[r][t] = max(K[r][t], (n + 127) // 128)
    for r in range(R):
        if min_one[r]:
            K[r] = np.maximum(K[r], 1)
    return K


def _build_stream(tiles, K, TLOC, r):
    """One core's range-r stream: padded idx/S arrays + block-end fix."""
    idx_l, dl_l, tile_ends = [], [], []
    pos = 0
    for t in range(TLOC):
        s_, d_ = tiles[t][r]
        slots = K[t] * 128
        n = s_.shape[0]
        assert n <= slots, (n, slots)
        si = np.zeros(slots, np.int64)
        di = np.full(slots, -1.0, np.float32)
        si[:n] = s_
        di[:n] = d_.astype(np.float32)
        idx_l.append(si)
        dl_l.append(di)
        pos += slots
        tile_ends.append(pos)
    if not idx_l or pos == 0:
        return (np.zeros((128, 1), np.int16),
                np.zeros((128, 1), np.uint8).view(f8),
                np.zeros((128, 1), np.float32).astype(bf))
    idx = np.concatenate(idx_l)
    dl = np.concatenate(dl_l)
    Ltot = idx.shape[0]

    # firmware trims trailing negative idxs per dma_gather: make sure the
    # last slot of every gather block is non-negative. Swap each negative
    # block-end slot with a distinct non-negative slot of the same dst
    # tile (slot order within a tile is free; S follows the final order).
    ends = list(range(GC * 128 - 1, Ltot, GC * 128))
    if not ends or ends[-1] != Ltot - 1:
        ends.append(Ltot - 1)
    ends_set = set(ends)
    te = np.asarray(tile_ends)
    ts = te - np.asarray([te[0]] + list(np.diff(te)))
    for t in range(len(te)):
        t0, t1 = int(ts[t]), int(te[t])
        needy = [p for p in ends if t0 <= p < t1 and idx[p] < 0]
        if not needy:
            continue
        donors = (q for q in range(t1 - 1, t0 - 1, -1)
                  if idx[q] >= 0 and q not in ends_set)
        for p in needy:
            q = next(donors)
            idx[p], idx[q] = idx[q], idx[p]
            dl[p], dl[q] = dl[q], dl[p]

    # one-hot S in fp8 bytes: [slot(128), chunk, 128]; dl as [slot, chunk]
    C = Ltot // 128
    dlm = dl.reshape(C, 128).astype(np.int32)
    if S_MODE == "stream":
        S = (dlm[:, :, None] == np.arange(128, dtype=np.int32))
        S = (S.astype(np.uint8) * 0x38).transpose(1, 0, 2).reshape(128, C * 128)
        S = np.ascontiguousarray(S).view(f8)
    else:
        S = np.zeros((128, 1), np.uint8).view(f8)
    dlc = np.ascontiguousarray(dl.reshape(C, 128).T).astype(bf)
    return _pack_idx(idx), S, dlc


def _cols_from_vec(v_padded, TL):
    """[TL*128] -> [128, TL] per-tile columns."""
    return np.ascontiguousarray(v_padded.reshape(TL, 128).T)


# ---------------------------------------------------------------- builder

def _build_program(NP, SLICE, l2rows, K1, K2, bounds,
                   n_cores_total, group_size):
    TLOC = SLICE // 128
    NL2 = len(K2)
    C1 = int(sum(K1))
    C2 = [int(sum(k)) for k in K2]
    MID1 = NP // 2

    nc = bacc.Bacc("TRN2", target_bir_lowering=False, debug=False,
                   num_devices=n_cores_total, num_swdge_queues=NQ,
                   dynamic_dma_scratch_size=65536)

    _sw = (lambda C: max(C * 128, 1)) if S_MODE == "stream" else (lambda C: 1)
    xs = nc.dram_tensor("xs", [NP, 128], BF16, kind="ExternalInput")
    rwrT = nc.dram_tensor("rwrT", [128, SLICE], BF16, kind="ExternalInput")
    xTloc = nc.dram_tensor("xTloc", [128, SLICE], BF16, kind="ExternalInput")
    idx1 = nc.dram_tensor("idx1", [128, max(C1 * 8, 1)], I16, kind="ExternalInput")
    S1 = nc.dram_tensor("S1", [128, _sw(C1)], FP8, kind="ExternalInput")
    idx2 = [nc.dram_tensor(f"idx2{r}", [128, max(C2[r] * 8, 1)], I16,
                           kind="ExternalInput") for r in range(NL2)]
    S2 = [nc.dram_tensor(f"S2{r}", [128, _sw(C2[r])], FP8,
                         kind="ExternalInput") for r in range(NL2)]
    dl1 = nc.dram_tensor("dl1", [128, max(C1, 1)], BF16, kind="ExternalInput")
    dl2 = [nc.dram_tensor(f"dl2{r}", [128, max(C2[r], 1)], BF16,
                          kind="ExternalInput") for r in range(NL2)]
    iota = nc.dram_tensor("iota", [128, 128], BF16, kind="ExternalInput")
    dinv_loc = nc.dram_tensor("dinv_loc", [128, TLOC], F32, kind="ExternalInput")
    dinvsq_loc = nc.dram_tensor("dinvsq_loc", [128, TLOC], F32, kind="ExternalInput")
    W12 = nc.dram_tensor("W12", [128, 128], BF16, kind="ExternalInput")
    linW = nc.dram_tensor("linW", [128, 128], BF16, kind="ExternalInput")
    combWt = nc.dram_tensor("combWt", [128, 128], BF16, kind="ExternalInput")
    combWb = nc.dram_tensor("combWb", [128, 128], BF16, kind="ExternalInput")
    ident = nc.dram_tensor("ident", [128, 128], BF16, kind="ExternalInput")
    emd_out = nc.dram_tensor("emd_out", [SLICE, 128], BF16, kind="ExternalOutput")

    groups = [
        list(range(g * group_size, (g + 1) * group_size))
        for g in range(n_cores_total // group_size)
    ]

    with tile.TileContext(nc) as tc:
        with tc.tile_pool(name="dram", bufs=1, space="DRAM") as dram, \
             tc.tile_pool(name="const", bufs=1) as cp, \
             tc.tile_pool(name="blkA", bufs=5) as bap, \
             tc.tile_pool(name="blkB", bufs=3) as bbp, \
             tc.tile_pool(name="sA", bufs=5) as sap, \
             tc.tile_pool(name="sB", bufs=3) as sbp, \
             tc.tile_pool(name="work", bufs=4) as wp, \
             tc.tile_pool(name="norm", bufs=4) as npools, \
             tc.tile_pool(name="ps_agg", bufs=3, space="PSUM") as ps_agg, \
             tc.tile_pool(name="ps_aux", bufs=2, space="PSUM") as ps_aux, \
             tc.tile_pool(name="ps_tr", bufs=2, space="PSUM") as ps_tr:

            h2slice = dram.tile([SLICE, 128], BF16)
            h2full = dram.tile([NP, 128], BF16)
            posT_d = dram.tile([SLICE, 128], BF16)

            def cload(t_dram, shape, dt, tag):
                t_sb = cp.tile(shape, dt, tag=tag)
                nc.sync.dma_start(t_sb[:], t_dram[:, :])
                return t_sb

            idx1_t = cload(idx1, [128, max(C1 * 8, 1)], I16, "idx1")
            idx2_t = [cload(idx2[r], [128, max(C2[r] * 8, 1)], I16, f"idx2{r}")
                      for r in range(NL2)]
            dl1_t = cload(dl1, [128, max(C1, 1)], BF16, "dl1")
            dl2_t = [cload(dl2[r], [128, max(C2[r], 1)], BF16, f"dl2{r}")
                     for r in range(NL2)]
            iota_t = cload(iota, [128, 128], BF16, "iota")
            dinvl_t = cload(dinv_loc, [128, TLOC], F32, "dinvl")
            dinvsq_t = cload(dinvsq_loc, [128, TLOC], F32, "dinvsq")
            W12_t = cload(W12, [128, 128], BF16, "W12")
            linW_t = cload(linW, [128, 128], BF16, "linW")
            combWt_t = cload(combWt, [128, 128], BF16, "combWt")
            combWb_t = cload(combWb, [128, 128], BF16, "combWb")
            ident_t = cload(ident, [128, 128], BF16, "ident")

            Copy = mybir.ActivationFunctionType.Copy

            def l1norm_scale(src_ap, out_tile_ap):
                """out = src / max(sum|src|, EPS), per-partition rows."""
                s_sum = npools.tile([128, 1], F32, tag="nsum")
                nc.vector.reduce_sum(
                    s_sum[:], src_ap, axis=mybir.AxisListType.X,
                    apply_absolute_value=True)
                s_max = npools.tile([128, 1], F32, tag="nmax")
                nc.vector.tensor_scalar_max(s_max[:], s_sum[:], EPS)
                r = npools.tile([128, 1], F32, tag="nrec")
                nc.vector.reciprocal(r[:], s_max[:])
                nc.scalar.activation(out_tile_ap, src_ap, Copy, scale=r[:, 0:1])

            qctr = [0]

            def agg_pass(streams, node_major):
                """Chunked aggregation over all local tiles.

                streams: list of (idx_sb, S_dram, dl_sb, table_ap, K, CT,
                bpool, spool, tag). node_major False: psum[f, dst]
                (lhsT=Hg, rhs=S); True: psum[dst, f] (lhsT=S, rhs=Hg).
                Yields (t, psum_tile) at each tile close.
                """
                issued = [0] * len(streams)
                blocks = [dict() for _ in streams]
                qpos = [0] * len(streams)

                def issue_block(r):
                    (idx_t, S_d, dl_t, table_ap, K, CT, bpool, spool,
                     tag) = streams[r]
                    b = issued[r]
                    q0 = b * GC
                    if q0 >= CT:
                        return
                    cb = min(GC, CT - q0)
                    blk = bpool.tile([128, GC, 128], BF16, tag="b" + tag)
                    nc.gpsimd.dma_gather(
                        blk[:, :cb, :], table_ap,
                        idx_t[:, q0 * 8:(q0 + cb) * 8],
                        num_idxs=cb * 128, num_idxs_reg=cb * 128,
                        elem_size=128, single_packet=SP,
                        queue_num=GATHER_QS[qctr[0] % len(GATHER_QS)])
                    qctr[0] += 1
                    sblk = spool.tile([128, GC, 128], FP8, tag="s" + tag)
                    if S_MODE == "stream":
                        nc.scalar.dma_start(
                            sblk[:, :cb, :],
                            S_d[:, q0 * 128:(q0 + cb) * 128]
                            .rearrange("p (c d) -> p c d", c=cb))
                    else:
                        nc.vector.tensor_tensor(
                            out=sblk[:, :cb, :],
                            in0=iota_t[:].unsqueeze(1)
                                .broadcast_to([128, cb, 128]),
                            in1=dl_t[:, q0:q0 + cb].unsqueeze(2)
                                .broadcast_to([128, cb, 128]),
                            op=mybir.AluOpType.is_equal)
                    blocks[r].pop(b - 5, None)
                    blocks[r][b] = (blk, sblk)
                    issued[r] = b + 1

                for t in range(TLOC):
                    ps = ps_agg.tile([128, 128], F32, tag="agg")
                    done = 0
                    for r, st in enumerate(streams):
                        K = st[4]
                        q = qpos[r]
                        for i in range(K[t]):
                            while issued[r] * GC <= q:
                                issue_block(r)
                            blk, sblk = blocks[r][q // GC]
                            s_t = sblk[:, q % GC, :]
                            hg = blk[:, q % GC, :]
                            if node_major:
                                nc.tensor.matmul(ps[:], lhsT=s_t, rhs=hg,
                                                 start=(done == 0), stop=False)
                            else:
                                nc.tensor.matmul(ps[:], lhsT=hg, rhs=s_t,
                                                 start=(done == 0), stop=False)
                            q += 1
                            done += 1
                        qpos[r] = q
                    yield t, ps

            # ================= layer 1: feature-major agg of xs -> h~2 slice,
            # with the group AllGather pipelined chunk-by-chunk
            cc_next = 0
            st1 = [(idx1_t, S1, dl1_t, xs[MID1:NP, :], K1, C1, bap, sap,
                    "1")]
            for t, ps in agg_pass(st1, False):
                # self term (pre-W12): += ident^T @ (dinv*X_T)[:, own tile]
                xl = wp.tile([128, 128], BF16, tag="xl")
                nc.sync.dma_start(xl[:], xTloc[:, t * 128:(t + 1) * 128])
                nc.tensor.matmul(ps[:], lhsT=ident_t[:], rhs=xl[:],
                                 start=False, stop=True)
                # close: M[f, dst] -> h~2 tile = dinvsq * (W12^T M)^T
                M_sb = wp.tile([128, 128], BF16, tag="aggT")
                nc.scalar.activation(M_sb[:], ps[:], Copy)
                h2T_ps = ps_aux.tile([128, 128], F32, tag="mm")
                nc.tensor.matmul(h2T_ps[:], lhsT=W12_t[:], rhs=M_sb[:],
                                 start=True, stop=True)
                h2T_sb = wp.tile([128, 128], BF16, tag="h2Ts")
                nc.scalar.activation(h2T_sb[:], h2T_ps[:], Copy)
                h2_ps = ps_tr.tile([128, 128], BF16, tag="tr")
                nc.tensor.transpose(h2_ps[:], h2T_sb[:], ident_t[:])
                h2_sb = wp.tile([128, 128], BF16, tag="h2s")
                nc.scalar.activation(h2_sb[:], h2_ps[:], Copy,
                                     scale=dinvsq_t[:, t:t + 1])
                nc.sync.dma_start(h2slice[t * 128:(t + 1) * 128, :], h2_sb[:])
                # pos = l1norm(rwr @ linW), transposed; input-only, done here
                rw = wp.tile([128, 128], BF16, tag="rw")
                nc.sync.dma_start(rw[:], rwrT[:, t * 128:(t + 1) * 128])
                pos_ps = ps_aux.tile([128, 128], F32, tag="mm")
                nc.tensor.matmul(pos_ps[:], lhsT=rw[:], rhs=linW_t[:],
                                 start=True, stop=True)
                pos_bf = wp.tile([128, 128], BF16, tag="posbf")
                l1norm_scale(pos_ps[:], pos_bf[:])
                posT_ps = ps_tr.tile([128, 128], BF16, tag="tr")
                nc.tensor.transpose(posT_ps[:], pos_bf[:], ident_t[:])
                posT_sb = wp.tile([128, 128], BF16, tag="posT")
                nc.scalar.activation(posT_sb[:], posT_ps[:], Copy)
                nc.sync.dma_start(posT_d[t * 128:(t + 1) * 128, :],
                                  posT_sb[:])
                if t + 1 == bounds[cc_next + 1]:
                    rs, re = bounds[cc_next] * 128, bounds[cc_next + 1] * 128
                    nc.gpsimd.collective_compute(
                        "AllGather", mybir.AluOpType.bypass,
                        replica_groups=groups,
                        ins=[h2slice[rs:re, :].opt()],
                        outs=[h2full[group_size * rs:group_size * re, :].opt()])
                    cc_next += 1

            # ================= layer 2: node-major agg + head
            st2 = [
                (idx2_t[r], S2[r], dl2_t[r],
                 h2full[l2rows[r]:l2rows[r + 1], :],
                 K2[r], C2[r], (bap if r < 2 else bbp),
                 (sap if r < 2 else sbp), f"2{r}")
                for r in range(NL2)
            ]
            for t, ps in agg_pass(st2, True):
                # self-loop term: += h~2[own tile] (identity matmul)
                h2s = wp.tile([128, 128], BF16, tag="h2self")
                nc.sync.dma_start(h2s[:], h2slice[t * 128:(t + 1) * 128, :])
                nc.tensor.matmul(ps[:], lhsT=ident_t[:], rhs=h2s[:],
                                 start=False, stop=True)
                # g = l1norm(dinv * agg2)
                g_pre = wp.tile([128, 128], F32, tag="gpre")
                nc.scalar.activation(g_pre[:], ps[:], Copy,
                                     scale=dinvl_t[:, t:t + 1])
                g_bf = wp.tile([128, 128], BF16, tag="gbf")
                l1norm_scale(g_pre[:], g_bf[:])
                gT_ps = ps_tr.tile([128, 128], BF16, tag="tr")
                nc.tensor.transpose(gT_ps[:], g_bf[:], ident_t[:])
                gT_sb = wp.tile([128, 128], BF16, tag="gT")
                nc.scalar.activation(gT_sb[:], gT_ps[:], Copy)

                # emd = l1norm(concat(pos, g) @ combW); posT precomputed
                posT_sb = wp.tile([128, 128], BF16, tag="posT")
                nc.sync.dma_start(posT_sb[:],
                                  posT_d[t * 128:(t + 1) * 128, :])

                emd_ps = ps_aux.tile([128, 128], F32, tag="mm")
                nc.tensor.matmul(emd_ps[:], lhsT=posT_sb[:], rhs=combWt_t[:],
                                 start=True, stop=False)
                nc.tensor.matmul(emd_ps[:], lhsT=gT_sb[:], rhs=combWb_t[:],
                                 start=False, stop=True)
                emd_f = wp.tile([128, 128], BF16, tag="emdf")
                l1norm_scale(emd_ps[:], emd_f[:])
                nc.sync.dma_start(emd_out[t * 128:(t + 1) * 128, :], emd_f[:])

    nc.compile()
    return nc


# ---------------------------------------------------------------- kernel

def _l2_rows(bounds, group_size, TLOC):
    """Layer-2 stream row boundaries: exchange-chunk-aligned spans of
    <= 32768 rows each; the first two exchange chunks get their own
    streams so layer-2 gathers start right after the first exchange."""
    rows = [group_size * 128 * b for b in bounds]
    NP = rows[-1]
    cuts = [rows[0]]
    for i, r in enumerate(rows[1:], 1):
        nxt = rows[i + 1] if i + 1 < len(rows) else None
        if len(cuts) < 3 and r - cuts[-1] > 0 and r < NP:
            cuts.append(r)
        elif nxt is None:
            cuts.append(r)
        elif nxt - cuts[-1] > 32768:
            cuts.append(r)
    if cuts[-1] != NP:
        cuts.append(NP)
    cuts = sorted(set(cuts))
    for a, b in zip(cuts[:-1], cuts[1:]):
        assert 0 < b - a <= 32768, (cuts, a, b)
    return cuts


def _run(inputs, N, E, n_cores_total=8, group_size=4):
    n_groups = n_cores_total // group_size
    assert n_groups == 2
    SLICE = ((N + group_size * 128 - 1) // (group_size * 128)) * 128
    NP = SLICE * group_size
    TLOC = SLICE // 128

    bounds = _chunk_bounds(TLOC)
    perm = _perm_from_bounds(bounds, SLICE, NP, group_size)
    l2rows = _l2_rows(bounds, group_size, TLOC)

    MID1 = NP // 2
    ranges1 = [(0, NP, MID1)]
    NL2 = len(l2rows) - 1
    ranges2 = [(l2rows[r], l2rows[r + 1], l2rows[r]) for r in range(NL2)]

    graphs = []
    for g in range(2):
        ei = inputs["edge_index1" if g == 0 else "edge_index2"]
        dinv, dinvsq, cores = _prep_graph(ei, N, NP, SLICE, group_size,
                                          ranges1)
        _, _, cores2 = _prep_graph(ei, N, NP, SLICE, group_size,
                                   ranges2, src_map=perm)
        graphs.append((dinv, dinvsq, cores, cores2))

    (K1,) = _slot_counts([g[2] for g in graphs], TLOC, 1, [True])
    K2 = _slot_counts([g[3] for g in graphs], TLOC, NL2,
                      [True] + [False] * (NL2 - 1))

    key = (NP, SLICE, tuple(l2rows), tuple(K1),
           tuple(tuple(k) for k in K2),
           tuple(bounds), n_cores_total, group_size)
    if key not in _prog_cache:
        _prog_cache[key] = _build_program(
            NP, SLICE, l2rows, K1, K2, bounds, n_cores_total, group_size)
    nc = _prog_cache[key]

    ident_np = np.eye(128, dtype=np.float32).astype(bf)
    W1f = np.asarray(inputs["conv1_W"], np.float32)
    W2f = np.asarray(inputs["conv2_W"], np.float32)
    W12_np = (W1f @ W2f).astype(bf)
    linW_np = np.asarray(inputs["lin_W"], np.float32).astype(bf)
    combW = np.asarray(inputs["comb_W"], np.float32)
    combWt_np = combW[:128].astype(bf)
    combWb_np = combW[128:].astype(bf)

    in_maps = []
    for core in range(n_cores_total):
        g = core // group_size
        c = core % group_size
        dinv, dinvsq, cores, cores2 = graphs[g]
        x = np.asarray(inputs["x1" if g == 0 else "x2"], np.float32)
        rwr = np.asarray(inputs["rwr1_emd" if g == 0 else "rwr2_emd"],
                         np.float32)

        dinv_p = np.ones(NP, np.float32)
        dinv_p[:N] = dinv
        dinvsq_p = np.ones(NP, np.float32)
        dinvsq_p[:N] = dinvsq

        xs = np.zeros((NP, 128), np.float32)
        xs[:N] = x * dinv[:, None]
        rwrT = np.zeros((128, SLICE), np.float32)
        lo, hi = c * SLICE, min((c + 1) * SLICE, N)
        if hi > lo:
            rwrT[:, :hi - lo] = rwr[lo:hi].T
        sl = slice(c * SLICE, (c + 1) * SLICE)
        xTloc = np.zeros((128, SLICE), np.float32)
        if hi > lo:
            xTloc[:, :hi - lo] = (x[lo:hi] * dinv[lo:hi, None]).T

        i1, s1, d1 = _build_stream(cores[c], K1, TLOC, 0)
        im = {
            "xs": xs.astype(bf),
            "rwrT": rwrT.astype(bf),
            "xTloc": xTloc.astype(bf),
            "idx1": i1, "S1": s1, "dl1": d1,
            "iota": np.broadcast_to(
                np.arange(128, dtype=np.float32), (128, 128)).astype(bf),
            "dinv_loc": _cols_from_vec(dinv_p[sl], TLOC),
            "dinvsq_loc": _cols_from_vec(dinvsq_p[sl], TLOC),
            "W12": W12_np, "linW": linW_np,
            "combWt": combWt_np, "combWb": combWb_np,
            "ident": ident_np,
        }
        for r in range(NL2):
            i2, s2, d2 = _build_stream(cores2[c], K2[r], TLOC, r)
            im[f"idx2{r}"] = i2
            im[f"S2{r}"] = s2
            im[f"dl2{r}"] = d2
        in_maps.append(im)

    import os
    if os.environ.get("GCN_SIM"):
        from concourse.bass_interp import MultiCoreSim
        sim = MultiCoreSim(nc, num_cores=n_cores_total, trace=False,
                           require_finite=False, require_nnan=False)
        cores = list(sim.cores.values())
        for c, core_sim in enumerate(cores):
            for k, v in in_maps[c].items():
                core_sim.tensor(k)[:] = v
        sim.simulate(check_with_hw=False)

        class _R:
            results = [{"emd_out": np.array(core_sim.tensor("emd_out"))}
                       for core_sim in cores]
        res = _R()
    else:
        trace = bool(os.environ.get("GCN_TRACE"))
        if trace:
            import sys, types
            if "antenv.axon_hooks" not in sys.modules:
                mod = types.ModuleType("antenv.axon_hooks")
                mod._hook = None
                mod.set_axon_ntff_profile_hook = \
                    lambda h: setattr(mod, "_hook", h)
                mod.get_axon_ntff_profile_hook = lambda: mod._hook
                sys.modules["antenv.axon_hooks"] = mod
                from trn_agent_boot.trn_boot import _ntff_profile_via_ctypes
                mod.set_axon_ntff_profile_hook(
                    _ntff_profile_via_ctypes('/opt/axon/libaxon_pjrt.so'))
        res = run_bass_kernel_spmd(nc, in_maps,
                                   core_ids=list(range(n_cores_total)),
                                   trace=trace)
        if trace:
            print(f"HW exec time: {res.exec_time_ns} ns "
                  f"(mean {res.mean_exec_time_ns}, "
                  f"core {res.max_exec_time_core_id})")
            if res.instructions_and_trace:
                print("trace:", res.instructions_and_trace[1])

    outs = []
    for g in range(2):
        parts = [np.asarray(res.results[g * group_size + c]["emd_out"],
                            np.float32) for c in range(group_size)]
        outs.append(np.concatenate(parts, axis=0)[:N])
    return outs[0], outs[1]


def kernel(rwr1_emd, rwr2_emd, x1, x2, edge_index1, edge_index2,
           lin_W, lin_b, conv1_W, conv1_b, conv2_W, conv2_b,
           comb_W, comb_b):
    for name, b in (("lin_b", lin_b), ("conv1_b", conv1_b),
                    ("conv2_b", conv2_b), ("comb_b", comb_b)):
        if np.any(np.asarray(b) != 0):
            raise NotImplementedError(f"nonzero bias {name} not supported")
    inputs = dict(rwr1_emd=rwr1_emd, rwr2_emd=rwr2_emd, x1=x1, x2=x2,
                  edge_index1=edge_index1, edge_index2=edge_index2,
                  lin_W=lin_W, conv1_W=conv1_W, conv2_W=conv2_W,
                  comb_W=comb_W)
    N = np.asarray(x1).shape[0]
    E = np.asarray(edge_index1).shape[1]
    return _run(inputs, N, E)


# revision 30
# speedup vs baseline: 1.2176x; 1.0090x over previous
"""Trainium2 Bass kernel for a 2-layer GCN pair (BRIGHT arch) on 8 NeuronCores.

Layout: cores 0-3 process graph 1, cores 4-7 process graph 2 (one SPMD
program; per-core inputs differ). Within a 4-core group each core owns a
contiguous slice of SLICE node rows (node space padded to NP rows).

v2 design (gather-descgen-bound, so everything serves the Q7):
- Layer-1 gathers read a host-prescaled table xs = dinv*x directly (no
  on-device stage-1 table build); W1@W2 is folded into one host matmul
  W12 applied per dst tile at close. Layer-1 indices are signed int16
  against the table midpoint (one merged stream; xs is an input so the
  out-of-AP-region reads carry no scheduling hazard). The firmware trims
  trailing negative indices per dma_gather, so block ends are host-fixed
  to be non-negative.
- The one-hot S routing matrices are host-precomputed in fp8 and
  streamed from DRAM (PE matmul takes bf16 lhsT x fp8 rhs), eliminating
  the DVE is_equal builds that interfered with Q7 descriptor writes.
- Layer-2 uses three unsigned streams split at h2-exchange chunk
  boundaries with exact in_ap regions, so each stream's gathers start as
  soon as its AllGather chunks land (pipelined against layer-1 closes;
  chunk-major h2full layout via host-permuted layer-2 indices).
- dma_gather blocks rotate across all 4 SWDGE queues.
"""

import numpy as np
import ml_dtypes

import concourse.bass as bass
import concourse.tile as tile
from concourse import bacc, mybir
from concourse.bass_utils import run_bass_kernel_spmd

F32 = mybir.dt.float32
BF16 = mybir.dt.bfloat16
FP8 = mybir.dt.float8e4
I16 = mybir.dt.int16

EPS = 1e-12
GC = 8   # gather-block size in chunks (GC*128 indices per dma_gather)
SP = True  # single-packet drains (safe at <=64+1 descs/engine, i.e. GC<=8)
S_MODE = "dve"  # "stream": host-built fp8 S from DRAM; "dve": is_equal builds
NQ = 4   # SWDGE queues
GATHER_QS = (0, 1, 2, 3)

_prog_cache: dict = {}

bf = ml_dtypes.bfloat16
f8 = ml_dtypes.float8_e4m3


# ---------------------------------------------------------------- host prep

def _chunk_bounds(TLOC):
    """Tile-index boundaries for the chunked h~2 AllGather pipeline."""
    if TLOC < 8:
        mid = (TLOC + 1) // 2
        return [0, mid, TLOC]
    fr = [0.0, 0.26, 0.52, 0.72, 0.88, 0.96, 1.0]
    b = sorted(set(int(round(f * TLOC)) for f in fr))
    return b


def _perm_from_bounds(bounds, SLICE, NP, group_size):
    """Node id -> row in the chunk-major h2full table."""
    p = np.empty(NP, np.int64)
    for s, e in zip(bounds[:-1], bounds[1:]):
        rs, re = s * 128, e * 128
        n_rows = re - rs
        for g in range(group_size):
            lo = g * SLICE + rs
            p[lo:lo + n_rows] = (group_size * rs + g * n_rows
                                 + np.arange(n_rows))
    return p


def _pack_idx(flat):
    """slot j -> wrapped [16, n/16] then replicated to [128, n/16] int16."""
    n = flat.shape[0]
    assert n % 16 == 0
    w = flat.reshape(n // 16, 16).T.astype(np.int16)
    return np.tile(w, (8, 1))


def _prep_graph(edge_index, N, NP, SLICE, n_cores, ranges, src_map=None):
    """Per-core, per-tile, per-range edge buckets for one graph.

    ranges: list of (lo, hi, base) row intervals partitioning [0, NP); an
    edge goes to the range containing its (mapped) src row, with index
    row - base. Returns (dinv, dinvsq, cores) with
    cores[c][t][r] = (idx_rel, dloc), idx_rel ascending.
    """
    src = np.asarray(edge_index[0], dtype=np.int64)
    dst = np.asarray(edge_index[1], dtype=np.int64)
    deg = np.bincount(dst, minlength=N).astype(np.float64) + 1.0
    dinv = (1.0 / np.sqrt(deg)).astype(np.float32)
    dinvsq = (1.0 / deg).astype(np.float32)

    rows = src_map[src] if src_map is not None else src
    for lo, hi, base in ranges:
        assert base - lo <= 32768 and hi - base <= 32768, (lo, hi, base)

    TLOC = SLICE // 128
    cores = []
    for c in range(n_cores):
        clo, chi = c * SLICE, (c + 1) * SLICE
        sel = (dst >= clo) & (dst < chi)
        s = rows[sel]
        d = dst[sel] - clo
        t_id = d // 128
        dloc = d % 128
        order = np.argsort(t_id, kind="stable")
        s, dloc, t_id = s[order], dloc[order], t_id[order]
        tiles = []
        for t in range(TLOC):
            m = t_id == t
            st, dt_ = s[m], dloc[m]
            bks = []
            for lo, hi, base in ranges:
                mr = (st >= lo) & (st < hi)
                sr = st[mr] - base
                dr = dt_[mr]
                o = np.argsort(sr, kind="stable")
                bks.append((sr[o].astype(np.int64), dr[o]))
            tiles.append(bks)
        cores.append(tiles)
    return dinv, dinvsq, cores


def _slot_counts(graph_cores_list, TLOC, R, min_one):
    """Shared per-tile per-range chunk counts (max across all datasets)."""
    K = [np.zeros(TLOC, np.int64) for _ in range(R)]
    for cores in graph_cores_list:
        for tiles in cores:
            for t in range(TLOC):
                for r in range(R):
                    n = len(tiles[t][r][0])
                    K[r][t] = max(K[r][t], (n + 127) // 128)
    for r in range(R):
        if min_one[r]:
            K[r] = np.maximum(K[r], 1)
    return K


def _build_stream(tiles, K, TLOC, r):
    """One core's range-r stream: padded idx/S arrays + block-end fix."""
    idx_l, dl_l, tile_ends = [], [], []
    pos = 0
    for t in range(TLOC):
        s_, d_ = tiles[t][r]
        slots = K[t] * 128
        n = s_.shape[0]
        assert n <= slots, (n, slots)
        si = np.zeros(slots, np.int64)
        di = np.full(slots, -1.0, np.float32)
        si[:n] = s_
        di[:n] = d_.astype(np.float32)
        idx_l.append(si)
        dl_l.append(di)
        pos += slots
        tile_ends.append(pos)
    if not idx_l or pos == 0:
        return (np.zeros((128, 1), np.int16),
                np.zeros((128, 1), np.uint8).view(f8),
                np.zeros((128, 1), np.float32).astype(bf))
    idx = np.concatenate(idx_l)
    dl = np.concatenate(dl_l)
    Ltot = idx.shape[0]

    # firmware trims trailing negative idxs per dma_gather: make sure the
    # last slot of every gather block is non-negative. Swap each negative
    # block-end slot with a distinct non-negative slot of the same dst
    # tile (slot order within a tile is free; S follows the final order).
    ends = list(range(GC * 128 - 1, Ltot, GC * 128))
    if not ends or ends[-1] != Ltot - 1:
        ends.append(Ltot - 1)
    ends_set = set(ends)
    te = np.asarray(tile_ends)
    ts = te - np.asarray([te[0]] + list(np.diff(te)))
    for t in range(len(te)):
        t0, t1 = int(ts[t]), int(te[t])
        needy = [p for p in ends if t0 <= p < t1 and idx[p] < 0]
        if not needy:
            continue
        donors = (q for q in range(t1 - 1, t0 - 1, -1)
                  if idx[q] >= 0 and q not in ends_set)
        for p in needy:
            q = next(donors)
            idx[p], idx[q] = idx[q], idx[p]
            dl[p], dl[q] = dl[q], dl[p]

    # one-hot S in fp8 bytes: [slot(128), chunk, 128]; dl as [slot, chunk]
    C = Ltot // 128
    dlm = dl.reshape(C, 128).astype(np.int32)
    if S_MODE == "stream":
        S = (dlm[:, :, None] == np.arange(128, dtype=np.int32))
        S = (S.astype(np.uint8) * 0x38).transpose(1, 0, 2).reshape(128, C * 128)
        S = np.ascontiguousarray(S).view(f8)
    else:
        S = np.zeros((128, 1), np.uint8).view(f8)
    dlc = np.ascontiguousarray(dl.reshape(C, 128).T).astype(bf)
    return _pack_idx(idx), S, dlc


def _cols_from_vec(v_padded, TL):
    """[TL*128] -> [128, TL] per-tile columns."""
    return np.ascontiguousarray(v_padded.reshape(TL, 128).T)


# ---------------------------------------------------------------- builder

def _build_program(NP, SLICE, l2rows, K1, K2, bounds,
                   n_cores_total, group_size):
    TLOC = SLICE // 128
    NL2 = len(K2)
    C1 = int(sum(K1))
    C2 = [int(sum(k)) for k in K2]
    MID1 = NP // 2

    nc = bacc.Bacc("TRN2", target_bir_lowering=False, debug=False,
                   num_devices=n_cores_total, num_swdge_queues=NQ,
                   dynamic_dma_scratch_size=65536)

    _sw = (lambda C: max(C * 128, 1)) if S_MODE == "stream" else (lambda C: 1)
    xs = nc.dram_tensor("xs", [NP, 128], BF16, kind="ExternalInput")
    rwrT = nc.dram_tensor("rwrT", [128, SLICE], BF16, kind="ExternalInput")
    xTloc = nc.dram_tensor("xTloc", [128, SLICE], BF16, kind="ExternalInput")
    idx1 = nc.dram_tensor("idx1", [128, max(C1 * 8, 1)], I16, kind="ExternalInput")
    S1 = nc.dram_tensor("S1", [128, _sw(C1)], FP8, kind="ExternalInput")
    idx2 = [nc.dram_tensor(f"idx2{r}", [128, max(C2[r] * 8, 1)], I16,
                           kind="ExternalInput") for r in range(NL2)]
    S2 = [nc.dram_tensor(f"S2{r}", [128, _sw(C2[r])], FP8,
                         kind="ExternalInput") for r in range(NL2)]
    dl1 = nc.dram_tensor("dl1", [128, max(C1, 1)], BF16, kind="ExternalInput")
    dl2 = [nc.dram_tensor(f"dl2{r}", [128, max(C2[r], 1)], BF16,
                          kind="ExternalInput") for r in range(NL2)]
    iota = nc.dram_tensor("iota", [128, 128], BF16, kind="ExternalInput")
    dinv_loc = nc.dram_tensor("dinv_loc", [128, TLOC], F32, kind="ExternalInput")
    dinvsq_loc = nc.dram_tensor("dinvsq_loc", [128, TLOC], F32, kind="ExternalInput")
    W12 = nc.dram_tensor("W12", [128, 128], BF16, kind="ExternalInput")
    linW = nc.dram_tensor("linW", [128, 128], BF16, kind="ExternalInput")
    combWt = nc.dram_tensor("combWt", [128, 128], BF16, kind="ExternalInput")
    combWb = nc.dram_tensor("combWb", [128, 128], BF16, kind="ExternalInput")
    ident = nc.dram_tensor("ident", [128, 128], BF16, kind="ExternalInput")
    emd_out = nc.dram_tensor("emd_out", [SLICE, 128], F32, kind="ExternalOutput")

    groups = [
        list(range(g * group_size, (g + 1) * group_size))
        for g in range(n_cores_total // group_size)
    ]

    with tile.TileContext(nc) as tc:
        with tc.tile_pool(name="dram", bufs=1, space="DRAM") as dram, \
             tc.tile_pool(name="const", bufs=1) as cp, \
             tc.tile_pool(name="blkA", bufs=5) as bap, \
             tc.tile_pool(name="blkB", bufs=3) as bbp, \
             tc.tile_pool(name="sA", bufs=5) as sap, \
             tc.tile_pool(name="sB", bufs=3) as sbp, \
             tc.tile_pool(name="work", bufs=3) as wp, \
             tc.tile_pool(name="norm", bufs=6) as npools, \
             tc.tile_pool(name="ps_agg", bufs=3, space="PSUM") as ps_agg, \
             tc.tile_pool(name="ps_aux", bufs=3, space="PSUM") as ps_aux, \
             tc.tile_pool(name="ps_tr", bufs=3, space="PSUM") as ps_tr:

            h2slice = dram.tile([SLICE, 128], BF16)
            h2full = dram.tile([NP, 128], BF16)
            posT_d = dram.tile([SLICE, 128], BF16)

            def cload(t_dram, shape, dt, tag):
                t_sb = cp.tile(shape, dt, tag=tag)
                nc.sync.dma_start(t_sb[:], t_dram[:, :])
                return t_sb

            idx1_t = cload(idx1, [128, max(C1 * 8, 1)], I16, "idx1")
            idx2_t = [cload(idx2[r], [128, max(C2[r] * 8, 1)], I16, f"idx2{r}")
                      for r in range(NL2)]
            dl1_t = cload(dl1, [128, max(C1, 1)], BF16, "dl1")
            dl2_t = [cload(dl2[r], [128, max(C2[r], 1)], BF16, f"dl2{r}")
                     for r in range(NL2)]
            iota_t = cload(iota, [128, 128], BF16, "iota")
            dinvl_t = cload(dinv_loc, [128, TLOC], F32, "dinvl")
            dinvsq_t = cload(dinvsq_loc, [128, TLOC], F32, "dinvsq")
            W12_t = cload(W12, [128, 128], BF16, "W12")
            linW_t = cload(linW, [128, 128], BF16, "linW")
            combWt_t = cload(combWt, [128, 128], BF16, "combWt")
            combWb_t = cload(combWb, [128, 128], BF16, "combWb")
            ident_t = cload(ident, [128, 128], BF16, "ident")

            Copy = mybir.ActivationFunctionType.Copy

            def l1norm_scale(src_ap, out_tile_ap):
                """out = src / max(sum|src|, EPS), per-partition rows."""
                s_sum = npools.tile([128, 1], F32, tag="nsum")
                nc.vector.reduce_sum(
                    s_sum[:], src_ap, axis=mybir.AxisListType.X,
                    apply_absolute_value=True)
                s_max = npools.tile([128, 1], F32, tag="nmax")
                nc.vector.tensor_scalar_max(s_max[:], s_sum[:], EPS)
                r = npools.tile([128, 1], F32, tag="nrec")
                nc.vector.reciprocal(r[:], s_max[:])
                nc.scalar.activation(out_tile_ap, src_ap, Copy, scale=r[:, 0:1])

            qctr = [0]

            def agg_pass(streams, node_major):
                """Chunked aggregation over all local tiles.

                streams: list of (idx_sb, S_dram, dl_sb, table_ap, K, CT,
                bpool, spool, tag). node_major False: psum[f, dst]
                (lhsT=Hg, rhs=S); True: psum[dst, f] (lhsT=S, rhs=Hg).
                Yields (t, psum_tile) at each tile close.
                """
                issued = [0] * len(streams)
                blocks = [dict() for _ in streams]
                qpos = [0] * len(streams)

                def issue_block(r):
                    (idx_t, S_d, dl_t, table_ap, K, CT, bpool, spool,
                     tag) = streams[r]
                    b = issued[r]
                    q0 = b * GC
                    if q0 >= CT:
                        return
                    cb = min(GC, CT - q0)
                    blk = bpool.tile([128, GC, 128], BF16, tag="b" + tag)
                    nc.gpsimd.dma_gather(
                        blk[:, :cb, :], table_ap,
                        idx_t[:, q0 * 8:(q0 + cb) * 8],
                        num_idxs=cb * 128, num_idxs_reg=cb * 128,
                        elem_size=128, single_packet=SP,
                        queue_num=GATHER_QS[qctr[0] % len(GATHER_QS)])
                    qctr[0] += 1
                    sblk = spool.tile([128, GC, 128], FP8, tag="s" + tag)
                    if S_MODE == "stream":
                        nc.scalar.dma_start(
                            sblk[:, :cb, :],
                            S_d[:, q0 * 128:(q0 + cb) * 128]
                            .rearrange("p (c d) -> p c d", c=cb))
                    else:
                        nc.vector.tensor_tensor(
                            out=sblk[:, :cb, :],
                            in0=iota_t[:].unsqueeze(1)
                                .broadcast_to([128, cb, 128]),
                            in1=dl_t[:, q0:q0 + cb].unsqueeze(2)
                                .broadcast_to([128, cb, 128]),
                            op=mybir.AluOpType.is_equal)
                    blocks[r].pop(b - 5, None)
                    blocks[r][b] = (blk, sblk)
                    issued[r] = b + 1

                for t in range(TLOC):
                    ps = ps_agg.tile([128, 128], F32, tag="agg")
                    done = 0
                    for r, st in enumerate(streams):
                        K = st[4]
                        q = qpos[r]
                        for i in range(K[t]):
                            while issued[r] * GC <= q:
                                issue_block(r)
                            blk, sblk = blocks[r][q // GC]
                            s_t = sblk[:, q % GC, :]
                            hg = blk[:, q % GC, :]
                            if node_major:
                                nc.tensor.matmul(ps[:], lhsT=s_t, rhs=hg,
                                                 start=(done == 0), stop=False)
                            else:
                                nc.tensor.matmul(ps[:], lhsT=hg, rhs=s_t,
                                                 start=(done == 0), stop=False)
                            q += 1
                            done += 1
                        qpos[r] = q
                    yield t, ps

            # ================= layer 1: feature-major agg of xs -> h~2 slice,
            # with the group AllGather pipelined chunk-by-chunk
            cc_next = 0
            st1 = [(idx1_t, S1, dl1_t, xs[MID1:NP, :], K1, C1, bap, sap,
                    "1")]
            for t, ps in agg_pass(st1, False):
                # self term (pre-W12): += ident^T @ (dinv*X_T)[:, own tile]
                xl = wp.tile([128, 128], BF16, tag="xl")
                nc.sync.dma_start(xl[:], xTloc[:, t * 128:(t + 1) * 128])
                nc.tensor.matmul(ps[:], lhsT=ident_t[:], rhs=xl[:],
                                 start=False, stop=True)
                # close: M[f, dst] -> h~2 tile = dinvsq * (W12^T M)^T
                M_sb = wp.tile([128, 128], BF16, tag="aggT")
                nc.scalar.activation(M_sb[:], ps[:], Copy)
                h2T_ps = ps_aux.tile([128, 128], F32, tag="mm")
                nc.tensor.matmul(h2T_ps[:], lhsT=W12_t[:], rhs=M_sb[:],
                                 start=True, stop=True)
                h2T_sb = wp.tile([128, 128], BF16, tag="h2Ts")
                nc.scalar.activation(h2T_sb[:], h2T_ps[:], Copy)
                h2_ps = ps_tr.tile([128, 128], BF16, tag="tr")
                nc.tensor.transpose(h2_ps[:], h2T_sb[:], ident_t[:])
                h2_sb = wp.tile([128, 128], BF16, tag="h2s")
                nc.scalar.activation(h2_sb[:], h2_ps[:], Copy,
                                     scale=dinvsq_t[:, t:t + 1])
                nc.sync.dma_start(h2slice[t * 128:(t + 1) * 128, :], h2_sb[:])
                # pos = l1norm(rwr @ linW), transposed; input-only, done here
                rw = wp.tile([128, 128], BF16, tag="rw")
                nc.sync.dma_start(rw[:], rwrT[:, t * 128:(t + 1) * 128])
                pos_ps = ps_aux.tile([128, 128], F32, tag="mm")
                nc.tensor.matmul(pos_ps[:], lhsT=rw[:], rhs=linW_t[:],
                                 start=True, stop=True)
                pos_bf = wp.tile([128, 128], BF16, tag="posbf")
                l1norm_scale(pos_ps[:], pos_bf[:])
                posT_ps = ps_tr.tile([128, 128], BF16, tag="tr")
                nc.tensor.transpose(posT_ps[:], pos_bf[:], ident_t[:])
                posT_sb = wp.tile([128, 128], BF16, tag="posT")
                nc.scalar.activation(posT_sb[:], posT_ps[:], Copy)
                nc.sync.dma_start(posT_d[t * 128:(t + 1) * 128, :],
                                  posT_sb[:])
                if t + 1 == bounds[cc_next + 1]:
                    rs, re = bounds[cc_next] * 128, bounds[cc_next + 1] * 128
                    nc.gpsimd.collective_compute(
                        "AllGather", mybir.AluOpType.bypass,
                        replica_groups=groups,
                        ins=[h2slice[rs:re, :].opt()],
                        outs=[h2full[group_size * rs:group_size * re, :].opt()])
                    cc_next += 1

            # ================= layer 2: node-major agg + head
            st2 = [
                (idx2_t[r], S2[r], dl2_t[r],
                 h2full[l2rows[r]:l2rows[r + 1], :],
                 K2[r], C2[r], (bap if r < 2 else bbp),
                 (sap if r < 2 else sbp), f"2{r}")
                for r in range(NL2)
            ]
            for t, ps in agg_pass(st2, True):
                # self-loop term: += h~2[own tile] (identity matmul)
                h2s = wp.tile([128, 128], BF16, tag="h2self")
                nc.sync.dma_start(h2s[:], h2slice[t * 128:(t + 1) * 128, :])
                nc.tensor.matmul(ps[:], lhsT=ident_t[:], rhs=h2s[:],
                                 start=False, stop=True)
                # g = l1norm(dinv * agg2)
                g_pre = wp.tile([128, 128], F32, tag="gpre")
                nc.scalar.activation(g_pre[:], ps[:], Copy,
                                     scale=dinvl_t[:, t:t + 1])
                g_bf = wp.tile([128, 128], BF16, tag="gbf")
                l1norm_scale(g_pre[:], g_bf[:])
                gT_ps = ps_tr.tile([128, 128], BF16, tag="tr")
                nc.tensor.transpose(gT_ps[:], g_bf[:], ident_t[:])
                gT_sb = wp.tile([128, 128], BF16, tag="gT")
                nc.scalar.activation(gT_sb[:], gT_ps[:], Copy)

                # emd = l1norm(concat(pos, g) @ combW); posT precomputed
                posT_sb = wp.tile([128, 128], BF16, tag="posT")
                nc.sync.dma_start(posT_sb[:],
                                  posT_d[t * 128:(t + 1) * 128, :])

                emd_ps = ps_aux.tile([128, 128], F32, tag="mm")
                nc.tensor.matmul(emd_ps[:], lhsT=posT_sb[:], rhs=combWt_t[:],
                                 start=True, stop=False)
                nc.tensor.matmul(emd_ps[:], lhsT=gT_sb[:], rhs=combWb_t[:],
                                 start=False, stop=True)
                emd_f = wp.tile([128, 128], F32, tag="emdf")
                l1norm_scale(emd_ps[:], emd_f[:])
                nc.sync.dma_start(emd_out[t * 128:(t + 1) * 128, :], emd_f[:])

    nc.compile()
    return nc


# ---------------------------------------------------------------- kernel

def _l2_rows(bounds, group_size, TLOC):
    """Layer-2 stream row boundaries: exchange-chunk-aligned spans of
    <= 32768 rows each; the first two exchange chunks get their own
    streams so layer-2 gathers start right after the first exchange."""
    rows = [group_size * 128 * b for b in bounds]
    NP = rows[-1]
    cuts = [rows[0]]
    for i, r in enumerate(rows[1:], 1):
        nxt = rows[i + 1] if i + 1 < len(rows) else None
        if len(cuts) < 3 and r - cuts[-1] > 0 and r < NP:
            cuts.append(r)
        elif nxt is None:
            cuts.append(r)
        elif nxt - cuts[-1] > 32768:
            cuts.append(r)
    if cuts[-1] != NP:
        cuts.append(NP)
    cuts = sorted(set(cuts))
    for a, b in zip(cuts[:-1], cuts[1:]):
        assert 0 < b - a <= 32768, (cuts, a, b)
    return cuts


def _run(inputs, N, E, n_cores_total=8, group_size=4):
    n_groups = n_cores_total // group_size
    assert n_groups == 2
    SLICE = ((N + group_size * 128 - 1) // (group_size * 128)) * 128
    NP = SLICE * group_size
    TLOC = SLICE // 128

    bounds = _chunk_bounds(TLOC)
    perm = _perm_from_bounds(bounds, SLICE, NP, group_size)
    l2rows = _l2_rows(bounds, group_size, TLOC)

    MID1 = NP // 2
    ranges1 = [(0, NP, MID1)]
    NL2 = len(l2rows) - 1
    ranges2 = [(l2rows[r], l2rows[r + 1], l2rows[r]) for r in range(NL2)]

    graphs = []
    for g in range(2):
        ei = inputs["edge_index1" if g == 0 else "edge_index2"]
        dinv, dinvsq, cores = _prep_graph(ei, N, NP, SLICE, group_size,
                                          ranges1)
        _, _, cores2 = _prep_graph(ei, N, NP, SLICE, group_size,
                                   ranges2, src_map=perm)
        graphs.append((dinv, dinvsq, cores, cores2))

    (K1,) = _slot_counts([g[2] for g in graphs], TLOC, 1, [True])
    K2 = _slot_counts([g[3] for g in graphs], TLOC, NL2,
                      [True] + [False] * (NL2 - 1))

    key = (NP, SLICE, tuple(l2rows), tuple(K1),
           tuple(tuple(k) for k in K2),
           tuple(bounds), n_cores_total, group_size)
    if key not in _prog_cache:
        _prog_cache[key] = _build_program(
            NP, SLICE, l2rows, K1, K2, bounds, n_cores_total, group_size)
    nc = _prog_cache[key]

    ident_np = np.eye(128, dtype=np.float32).astype(bf)
    W1f = np.asarray(inputs["conv1_W"], np.float32)
    W2f = np.asarray(inputs["conv2_W"], np.float32)
    W12_np = (W1f @ W2f).astype(bf)
    linW_np = np.asarray(inputs["lin_W"], np.float32).astype(bf)
    combW = np.asarray(inputs["comb_W"], np.float32)
    combWt_np = combW[:128].astype(bf)
    combWb_np = combW[128:].astype(bf)

    in_maps = []
    for core in range(n_cores_total):
        g = core // group_size
        c = core % group_size
        dinv, dinvsq, cores, cores2 = graphs[g]
        x = np.asarray(inputs["x1" if g == 0 else "x2"], np.float32)
        rwr = np.asarray(inputs["rwr1_emd" if g == 0 else "rwr2_emd"],
                         np.float32)

        dinv_p = np.ones(NP, np.float32)
        dinv_p[:N] = dinv
        dinvsq_p = np.ones(NP, np.float32)
        dinvsq_p[:N] = dinvsq

        xs = np.zeros((NP, 128), np.float32)
        xs[:N] = x * dinv[:, None]
        rwrT = np.zeros((128, SLICE), np.float32)
        lo, hi = c * SLICE, min((c + 1) * SLICE, N)
        if hi > lo:
            rwrT[:, :hi - lo] = rwr[lo:hi].T
        sl = slice(c * SLICE, (c + 1) * SLICE)
        xTloc = np.zeros((128, SLICE), np.float32)
        if hi > lo:
            xTloc[:, :hi - lo] = (x[lo:hi] * dinv[lo:hi, None]).T

        i1, s1, d1 = _build_stream(cores[c], K1, TLOC, 0)
        im = {
            "xs": xs.astype(bf),
            "rwrT": rwrT.astype(bf),
            "xTloc": xTloc.astype(bf),
            "idx1": i1, "S1": s1, "dl1": d1,
            "iota": np.broadcast_to(
                np.arange(128, dtype=np.float32), (128, 128)).astype(bf),
            "dinv_loc": _cols_from_vec(dinv_p[sl], TLOC),
            "dinvsq_loc": _cols_from_vec(dinvsq_p[sl], TLOC),
            "W12": W12_np, "linW": linW_np,
            "combWt": combWt_np, "combWb": combWb_np,
            "ident": ident_np,
        }
        for r in range(NL2):
            i2, s2, d2 = _build_stream(cores2[c], K2[r], TLOC, r)
            im[f"idx2{r}"] = i2
            im[f"S2{r}"] = s2
            im[f"dl2{r}"] = d2
        in_maps.append(im)

    import os
    if os.environ.get("GCN_SIM"):
        from concourse.bass_interp import MultiCoreSim
        sim = MultiCoreSim(nc, num_cores=n_cores_total, trace=False,
                           require_finite=False, require_nnan=False)
        cores = list(sim.cores.values())
        for c, core_sim in enumerate(cores):
            for k, v in in_maps[c].items():
                core_sim.tensor(k)[:] = v
        sim.simulate(check_with_hw=False)

        class _R:
            results = [{"emd_out": np.array(core_sim.tensor("emd_out"))}
                       for core_sim in cores]
        res = _R()
    else:
        trace = bool(os.environ.get("GCN_TRACE"))
        if trace:
            import sys, types
            if "antenv.axon_hooks" not in sys.modules:
                mod = types.ModuleType("antenv.axon_hooks")
                mod._hook = None
                mod.set_axon_ntff_profile_hook = \
                    lambda h: setattr(mod, "_hook", h)
                mod.get_axon_ntff_profile_hook = lambda: mod._hook
                sys.modules["antenv.axon_hooks"] = mod
                from trn_agent_boot.trn_boot import _ntff_profile_via_ctypes
                mod.set_axon_ntff_profile_hook(
                    _ntff_profile_via_ctypes('/opt/axon/libaxon_pjrt.so'))
        res = run_bass_kernel_spmd(nc, in_maps,
                                   core_ids=list(range(n_cores_total)),
                                   trace=trace)
        if trace:
            print(f"HW exec time: {res.exec_time_ns} ns "
                  f"(mean {res.mean_exec_time_ns}, "
                  f"core {res.max_exec_time_core_id})")
            if res.instructions_and_trace:
                print("trace:", res.instructions_and_trace[1])

    outs = []
    for g in range(2):
        parts = [res.results[g * group_size + c]["emd_out"]
                 for c in range(group_size)]
        outs.append(np.concatenate(parts, axis=0)[:N])
    return outs[0], outs[1]


def kernel(rwr1_emd, rwr2_emd, x1, x2, edge_index1, edge_index2,
           lin_W, lin_b, conv1_W, conv1_b, conv2_W, conv2_b,
           comb_W, comb_b):
    for name, b in (("lin_b", lin_b), ("conv1_b", conv1_b),
                    ("conv2_b", conv2_b), ("comb_b", comb_b)):
        if np.any(np.asarray(b) != 0):
            raise NotImplementedError(f"nonzero bias {name} not supported")
    inputs = dict(rwr1_emd=rwr1_emd, rwr2_emd=rwr2_emd, x1=x1, x2=x2,
                  edge_index1=edge_index1, edge_index2=edge_index2,
                  lin_W=lin_W, conv1_W=conv1_W, conv2_W=conv2_W,
                  comb_W=comb_W)
    N = np.asarray(x1).shape[0]
    E = np.asarray(edge_index1).shape[1]
    return _run(inputs, N, E)
